# revision 1
# baseline (speedup 1.0000x reference)
"""Trainium2 Bass kernel for CovariateAttention (B=2, S=2048, E=1024, 16 heads).

Sharding: 8 cores = 2 (batch) x 4 (head groups of 4 heads).
Per core: q/k/v projections for its 4 heads (tensor-parallel column shard),
RoPE, causal flash-style attention with transposed scores, output projection
row-shard producing a partial [S, E] result; host sums the 4 partials per batch.

Layout strategy (everything pre-transposed on host so no device transposes):
  xT   [E, S]    - x[b] transposed
  wqT/wkT [E, 256] - per-head-deinterleaved (RoPE pair permutation) W slices, transposed
  wvT  [E, 256]
  woT  [256, E]  - Wo column-slice transposed
  qT/kT on device: [d_local, S] with per-head layout [x1(32) | x2(32)]
  scoresT [k_pos, q_pos] so softmax sums come from a ones-row in the PV matmul.
Matmul operand dtype defaults to float32r (~2e-4 rel err, 241us); set
ANT_KERNEL_DTYPE=bf16 for ~220us at ~3e-3 rel err.
"""

import os
import sys

sys.path.insert(0, "/opt/trn_rl_repo")

import numpy as np

N_HEADS = 16
ROPE_BASE = 10000.0
B, S, E = 2, 2048, 1024
D_ATTN = 1024
HDIM = 64
HALF = HDIM // 2
GROUP_HEADS = 4          # heads per core
DL = GROUP_HEADS * HDIM  # 256 local dims per core
N_CORES = 8
ATTN_SCALE = 1.0 / np.sqrt(D_ATTN)

DTYPE_MODE = os.environ.get("ANT_KERNEL_DTYPE", "f32r")

_CACHE = {}


def _build_nc(mode):
    import concourse.tile as tile
    from concourse import bacc, mybir

    f32 = mybir.dt.float32
    dt = mybir.dt.bfloat16 if mode == "bf16" else mybir.dt.float32r
    da = mybir.dt.float32r if mode == "f32r" else mybir.dt.bfloat16  # attention core
    dt_bits = mybir.dt.uint16 if mode == "bf16" else mybir.dt.uint32

    nc = bacc.Bacc("TRN2", target_bir_lowering=False, debug=False, num_devices=N_CORES)

    xT_d = nc.dram_tensor("xT", [E, S], dt, kind="ExternalInput").ap()
    wqT_d = nc.dram_tensor("wqT", [E, DL], dt, kind="ExternalInput").ap()
    wkT_d = nc.dram_tensor("wkT", [E, DL], dt, kind="ExternalInput").ap()
    wvT_d = nc.dram_tensor("wvT", [E, DL], dt, kind="ExternalInput").ap()
    woT_d = nc.dram_tensor("woT", [DL, E], dt, kind="ExternalInput").ap()
    cos_d = nc.dram_tensor("cosP", [128, S], da, kind="ExternalInput").ap()
    sin_d = nc.dram_tensor("sinP", [128, S], da, kind="ExternalInput").ap()
    tri_d = nc.dram_tensor("tri", [128, 128], da, kind="ExternalInput").ap()
    ones_d = nc.dram_tensor("ones", [128, GROUP_HEADS], da, kind="ExternalInput").ap()
    part_d = nc.dram_tensor("part", [S, E], f32, kind="ExternalOutput").ap()

    NSB = 4    # s-blocks of 512 (projection phase)
    NET = 8    # e-tiles of 128 (contraction)
    NQB = 4    # q-blocks of 512 (attention phase)
    NKT = 16   # k-tiles of 128
    NQT = 16   # q-tiles of 128 (output projection)
    Exp = mybir.ActivationFunctionType.Exp

    with tile.TileContext(nc) as tc:
        with (
            tc.tile_pool(name="weights", bufs=1) as wpool,
            tc.tile_pool(name="persist", bufs=1) as persist,
            tc.tile_pool(name="xin", bufs=2) as xin,
            tc.tile_pool(name="rope", bufs=3 if mode == "bf16" else 2) as rope,
            tc.tile_pool(name="probs", bufs=4 if mode == "bf16" else 3) as probs,
            tc.tile_pool(name="small", bufs=3) as small,
            tc.tile_pool(name="fout", bufs=3 if mode == "bf16" else 2) as fopool,
            tc.tile_pool(name="sc_ps", bufs=2, space="PSUM") as sc_ps,
            tc.tile_pool(name="pv_ps", bufs=2, space="PSUM") as pv_ps,
        ):
            # ---- load weights/constants (resident) ----
            wq_sb = wpool.tile([128, NET, DL], dt, tag="wq")
            wk_sb = wpool.tile([128, NET, DL], dt, tag="wk")
            wv_sb = wpool.tile([128, NET, DL], dt, tag="wv")
            wo_sb = wpool.tile([128, 2, E], dt, tag="wo")
            cos_sb = wpool.tile([128, S], da, tag="cos")
            sin_sb = wpool.tile([128, S], da, tag="sin")
            tri_sb = wpool.tile([128, 128], da, tag="tri")
            nc.sync.dma_start(out=wq_sb[:], in_=wqT_d.rearrange("(t p) d -> p t d", p=128))
            nc.sync.dma_start(out=wk_sb[:], in_=wkT_d.rearrange("(t p) d -> p t d", p=128))
            nc.sync.dma_start(out=wv_sb[:], in_=wvT_d.rearrange("(t p) d -> p t d", p=128))
            nc.sync.dma_start(out=wo_sb[:], in_=woT_d.rearrange("(t p) e -> p t e", p=128))
            nc.sync.dma_start(out=cos_sb[:], in_=cos_d[:])
            nc.sync.dma_start(out=sin_sb[:], in_=sin_d[:])
            nc.sync.dma_start(out=tri_sb[:], in_=tri_d[:])

            # persistent activations
            qT = [persist.tile([128, S], da, tag=f"qT{t}", name=f"qT{t}") for t in range(2)]
            kT = [persist.tile([128, S], da, tag=f"kT{t}", name=f"kT{t}") for t in range(2)]
            outT = [persist.tile([128, S], dt, tag=f"outT{t}", name=f"outT{t}") for t in range(2)]
            vt = [persist.tile([128, GROUP_HEADS, HDIM + 1], da, tag=f"vt{i}", name=f"vt{i}")
                  for i in range(NKT)]

            # ---- Phase A: projections + RoPE (emitted per s-block) ----
            def phase_a(sb):
                ssl = slice(sb * 512, (sb + 1) * 512)
                x_sbs = []
                for eh in range(4):
                    x_q = xin.tile([128, 2, 512], dt, tag=f"x{eh}", name=f"x{eh}_{sb}")
                    nc.sync.dma_start(
                        out=x_q[:],
                        in_=xT_d[eh * 256:(eh + 1) * 256, ssl]
                        .rearrange("(t p) s -> p t s", p=128),
                    )
                    x_sbs.append(x_q)

                def xe(et):
                    return x_sbs[et // 2][:, et % 2, :]

                for dtl in range(2):
                    dsl = slice(dtl * 128, (dtl + 1) * 128)
                    for w_sb, dest in ((wq_sb, qT), (wk_sb, kT)):
                        pp = pv_ps.tile([128, 512], f32, tag="ppv", name=f"pp{sb}{dtl}")
                        for et in range(NET):
                            nc.tensor.matmul(
                                pp[:], w_sb[:, et, dsl], xe(et),
                                start=(et == 0), stop=(et == NET - 1),
                            )
                        # RoPE: r = raw*C + rot(raw)*S
                        raw = rope.tile([128, 512], da, tag="raw")
                        nc.vector.tensor_copy(raw[:], pp[:])
                        rot = rope.tile([128, 512], da, tag="rot")
                        for blk in range(4):
                            srcb = (blk ^ 1) * 32
                            nc.gpsimd.dma_start(
                                out=rot[blk * 32:(blk + 1) * 32, :],
                                in_=raw[srcb:srcb + 32, :],
                            )
                        t1 = rope.tile([128, 512], da, tag="t1")
                        nc.vector.tensor_mul(t1[:], raw[:], cos_sb[:, ssl])
                        t2 = rope.tile([128, 512], da, tag="t2")
                        nc.vector.tensor_mul(t2[:], rot[:], sin_sb[:, ssl])
                        nc.vector.tensor_add(dest[dtl][:, ssl], t1[:], t2[:])
                # v projection (natural layout [s, d_local]) + ones column
                for st in range(4):
                    kt = sb * 4 + st
                    vp = pv_ps.tile([128, DL], f32, tag="ppv", name=f"vp{kt}")
                    for et in range(NET):
                        nc.tensor.matmul(
                            vp[:], xe(et)[:, st * 128:(st + 1) * 128],
                            wv_sb[:, et, :],
                            start=(et == 0), stop=(et == NET - 1),
                        )
                    nc.vector.tensor_copy(
                        vt[kt][:, :, 0:HDIM],
                        vp.rearrange("p (h d) -> p h d", h=GROUP_HEADS),
                    )
                    nc.gpsimd.dma_start(
                        out=vt[kt][:, :, HDIM:HDIM + 1],
                        in_=ones_d.rearrange("p (h o) -> p h o", o=1),
                    )

            # ---- Phase B: attention (emitted per q-block) ----
            def phase_b(qb):
                qsl = slice(qb * 512, (qb + 1) * 512)
                for h in range(GROUP_HEADS):
                    t, base = h // 2, (h % 2) * 64
                    psl = slice(base, base + 64)
                    pv = pv_ps.tile([128, 512], f32, tag="ppv", name=f"pv{qb}{h}")
                    nkt = 4 * (qb + 1)
                    for kp in range(nkt // 2):
                        sc = sc_ps.tile([128, 1024], f32, tag="sc")
                        for j in range(2):
                            kt = 2 * kp + j
                            # diag k-tile: only columns >= o are causally valid
                            o = max(kt * 128 - qb * 512, 0)
                            nc.tensor.matmul(
                                sc[:, j * 512 + o:(j + 1) * 512],
                                kT[t][psl, kt * 128:(kt + 1) * 128],
                                qT[t][psl, qb * 512 + o:(qb + 1) * 512],
                                start=True, stop=True,
                            )
                        o0 = max(2 * kp * 128 - qb * 512, 0)
                        pr = probs.tile([128, 1024], da, tag="pr")
                        nc.scalar.activation(
                            pr[:, o0:], sc[:, o0:], Exp, scale=ATTN_SCALE
                        )
                        for j in range(2):
                            kt = 2 * kp + j
                            o = max(kt * 128 - qb * 512, 0)
                            if kt >= 4 * qb:  # diagonal k-tile: triangular mask
                                nc.vector.tensor_mul(
                                    pr[:, j * 512 + o:j * 512 + o + 128],
                                    pr[:, j * 512 + o:j * 512 + o + 128],
                                    tri_sb[:],
                                )
                            nc.tensor.matmul(
                                pv[0:65, o:512], vt[kt][:, h, :],
                                pr[:, j * 512 + o:(j + 1) * 512],
                                start=(kt == 0), stop=(kt == nkt - 1),
                            )
                    pvs = small.tile([65, 512], f32, tag="pvs", bufs=2)
                    nc.vector.tensor_copy(pvs[:], pv[0:65, :])
                    sums = small.tile([1, 512], f32, tag="sums")
                    nc.vector.tensor_copy(sums[:], pvs[64:65, :])
                    inv = small.tile([1, 512], f32, tag="inv")
                    nc.vector.reciprocal_approx_fast(out=inv[:], in_=sums[:])
                    invb = small.tile([64, 512], f32, tag="invb")
                    nc.gpsimd.partition_broadcast(invb[:], inv[:])
                    nc.vector.tensor_mul(
                        outT[t][base:base + 64, qsl], pvs[0:64, :], invb[:]
                    )

            # ---- Phase C: output projection (partial over local dims) ----
            def phase_c(qt):
                qsl = slice(qt * 128, (qt + 1) * 128)
                f = sc_ps.tile([128, 1024], f32, tag="fC", name=f"f{qt}", bufs=1)
                for eb in range(2):
                    esl = slice(eb * 512, (eb + 1) * 512)
                    for dtl in range(2):
                        nc.tensor.matmul(
                            f[:, esl], outT[dtl][:, qsl], wo_sb[:, dtl, esl],
                            start=(dtl == 0), stop=(dtl == 1),
                        )
                fo = fopool.tile([128, 1024], f32, tag="fo")
                if qt % 2 == 0:
                    nc.scalar.copy(fo[:], f[:])
                else:
                    nc.vector.tensor_copy(fo[:], f[:])
                nc.sync.dma_start(out=part_d[qsl, :], in_=fo[:])

            # ---- interleaved emission: A(sb) feeds B(qb=sb); C trails B ----
            phase_a(0)
            for blk in range(NQB):
                if blk + 1 < NSB:
                    phase_a(blk + 1)
                phase_b(blk)
                if blk >= 1:
                    for qt in range(4 * (blk - 1), 4 * blk):
                        phase_c(qt)
            for qt in range(4 * (NQB - 1), NQT):
                phase_c(qt)

    nc.compile()
    return nc


def _host_tables():
    inv_freq = 1.0 / (ROPE_BASE ** (np.arange(HALF, dtype=np.float32) / HALF))
    angles = np.arange(S, dtype=np.float32)[:, None] * inv_freq[None, :]  # [S, 32]
    cos = np.cos(angles).T.astype(np.float32)  # [32, S]
    sin = np.sin(angles).T.astype(np.float32)
    cosP = np.tile(cos, (4, 1))                                   # [128, S]
    sinP = np.concatenate([-sin, sin, -sin, sin], axis=0).astype(np.float32)
    tri = (np.arange(128)[None, :] >= np.arange(128)[:, None]).astype(np.float32)
    return cosP, sinP, np.ascontiguousarray(tri)


def kernel(x, Wq, Wk, Wv, Wo):
    import ml_dtypes
    from concourse.bass_utils import run_bass_kernel_spmd

    x = np.asarray(x, dtype=np.float32)
    Wq = np.asarray(Wq, dtype=np.float32)
    Wk = np.asarray(Wk, dtype=np.float32)
    Wv = np.asarray(Wv, dtype=np.float32)
    Wo = np.asarray(Wo, dtype=np.float32)

    mode = DTYPE_MODE
    if ("nc", mode) not in _CACHE:
        _CACHE[("nc", mode)] = _build_nc(mode)
    nc = _CACHE[("nc", mode)]

    np_dt = ml_dtypes.bfloat16 if mode == "bf16" else np.float32
    np_da = np.float32 if mode == "f32r" else ml_dtypes.bfloat16

    def cvt(a):
        return np.ascontiguousarray(a.astype(np_dt))

    def cva(a):
        return np.ascontiguousarray(a.astype(np_da))

    # RoPE pair deinterleave permutation within each head: [0,2,..62, 1,3,..63]
    perm = np.concatenate([np.arange(0, HDIM, 2), np.arange(1, HDIM, 2)])
    full_perm = np.concatenate([h * HDIM + perm for h in range(N_HEADS)])
    Wq_p = Wq[full_perm]
    Wk_p = Wk[full_perm]

    cosP, sinP, tri = _host_tables()
    xT = [cvt(x[b].T) for b in range(B)]
    cosP, sinP, tri = cva(cosP), cva(sinP), cva(tri)
    ones = np.ones((128, GROUP_HEADS), dtype=np_da)

    in_maps = []
    for c in range(N_CORES):
        b, g = c // 4, c % 4
        dsl = slice(g * DL, (g + 1) * DL)
        in_maps.append({
            "xT": xT[b],
            "wqT": cvt(Wq_p[dsl].T),
            "wkT": cvt(Wk_p[dsl].T),
            "wvT": cvt(Wv[dsl].T),
            "woT": cvt(Wo[:, dsl].T),
            "cosP": cosP,
            "sinP": sinP,
            "tri": tri,
            "ones": ones,
        })

    trace = bool(int(os.environ.get("ANT_KERNEL_TRACE", "0")))
    res = None
    for attempt in range(3):
        try:
            res = run_bass_kernel_spmd(
                nc, in_maps, core_ids=list(range(N_CORES)), trace=trace
            )
            break
        except Exception:
            if attempt == 2:
                raise
            import time as _time
            _time.sleep(20)
    _CACHE["last_exec_time_ns"] = res.exec_time_ns
    _CACHE["last_res"] = res

    out = np.zeros((B, S, E), dtype=np.float32)
    for c in range(N_CORES):
        out[c // 4] += res.results[c]["part"]
    return out



# revision 15
# speedup vs baseline: 1.1091x; 1.1091x over previous
"""Trainium2 Bass kernel for CovariateAttention (B=2, S=2048, E=1024, 16 heads).

Sharding: 8 cores = 2 (batch) x 4 (head groups of 4 heads).
Per core: q/k/v projections for its 4 heads (tensor-parallel column shard),
RoPE, causal flash-style attention with transposed scores, output projection
row-shard producing a partial [S, E] result; host sums the 4 partials per batch.

Layout strategy (everything pre-transposed on host so no device transposes):
  xT   [E, S]    - x[b] transposed
  wqT/wkT [E, 256] - per-head-deinterleaved (RoPE pair permutation) W slices, transposed
  wvT  [E, 256]
  woT  [256, E]  - Wo column-slice transposed
  qT/kT on device: [d_local, S] with per-head layout [x1(32) | x2(32)]
  scoresT [k_pos, q_pos] so softmax sums come from a ones-row in the PV matmul.
Matmul operand dtype defaults to float32r (~2e-4 rel err, 241us); set
ANT_KERNEL_DTYPE=bf16 for ~220us at ~3e-3 rel err.
"""

import os
import sys

sys.path.insert(0, "/opt/trn_rl_repo")

import numpy as np

N_HEADS = 16
ROPE_BASE = 10000.0
B, S, E = 2, 2048, 1024
D_ATTN = 1024
HDIM = 64
HALF = HDIM // 2
GROUP_HEADS = 4          # heads per core
DL = GROUP_HEADS * HDIM  # 256 local dims per core
N_CORES = 8
ATTN_SCALE = 1.0 / np.sqrt(D_ATTN)

DTYPE_MODE = os.environ.get("ANT_KERNEL_DTYPE", "bf16")

_CACHE = {}


def _build_nc(mode):
    import concourse.tile as tile
    from concourse import bacc, mybir

    f32 = mybir.dt.float32
    dt = mybir.dt.bfloat16 if mode == "bf16" else mybir.dt.float32r
    da = mybir.dt.float32r if mode == "f32r" else mybir.dt.bfloat16  # attention core
    dt_bits = mybir.dt.uint16 if mode == "bf16" else mybir.dt.uint32

    nc = bacc.Bacc("TRN2", target_bir_lowering=False, debug=False, num_devices=N_CORES)

    xT_d = nc.dram_tensor("xT", [E, S], dt, kind="ExternalInput").ap()
    wqT_d = nc.dram_tensor("wqT", [E, DL], dt, kind="ExternalInput").ap()
    wkT_d = nc.dram_tensor("wkT", [E, DL], dt, kind="ExternalInput").ap()
    wvT_d = nc.dram_tensor("wvT", [E, DL], dt, kind="ExternalInput").ap()
    woT_d = nc.dram_tensor("woT", [DL, E], dt, kind="ExternalInput").ap()
    cos_d = nc.dram_tensor("cosP", [128, S], da, kind="ExternalInput").ap()
    sin_d = nc.dram_tensor("sinP", [128, S], da, kind="ExternalInput").ap()
    tri_d = nc.dram_tensor("tri", [128, 128], da, kind="ExternalInput").ap()
    ones_d = nc.dram_tensor("ones", [128, GROUP_HEADS], da, kind="ExternalInput").ap()
    part_d = nc.dram_tensor("part", [S, E], f32, kind="ExternalOutput").ap()

    NSB = 4    # s-blocks of 512 (projection phase)
    NET = 8    # e-tiles of 128 (contraction)
    NQB = 4    # q-blocks of 512 (attention phase)
    NKT = 16   # k-tiles of 128
    NQT = 16   # q-tiles of 128 (output projection)
    Exp = mybir.ActivationFunctionType.Exp

    with tile.TileContext(nc) as tc:
        with (
            tc.tile_pool(name="weights", bufs=1) as wpool,
            tc.tile_pool(name="persist", bufs=1) as persist,
            tc.tile_pool(name="xin", bufs=2) as xin,
            tc.tile_pool(name="rope", bufs=3 if mode == "bf16" else 2) as rope,
            tc.tile_pool(name="probs", bufs=4 if mode == "bf16" else 3) as probs,
            tc.tile_pool(name="small", bufs=3) as small,
            tc.tile_pool(name="fout", bufs=3 if mode == "bf16" else 2) as fopool,
            tc.tile_pool(name="sc_ps", bufs=2, space="PSUM") as sc_ps,
            tc.tile_pool(name="pv_ps", bufs=2, space="PSUM") as pv_ps,
        ):
            # ---- load weights/constants (resident) ----
            wq_sb = wpool.tile([128, NET, DL], dt, tag="wq")
            wk_sb = wpool.tile([128, NET, DL], dt, tag="wk")
            wv_sb = wpool.tile([128, NET, DL], dt, tag="wv")
            wo_sb = wpool.tile([128, 2, E], dt, tag="wo")
            cos_sb = wpool.tile([128, S], da, tag="cos")
            sin_sb = wpool.tile([128, S], da, tag="sin")
            tri_sb = wpool.tile([128, 128], da, tag="tri")
            nc.sync.dma_start(out=wq_sb[:], in_=wqT_d.rearrange("(t p) d -> p t d", p=128))
            nc.sync.dma_start(out=wk_sb[:], in_=wkT_d.rearrange("(t p) d -> p t d", p=128))
            nc.sync.dma_start(out=wv_sb[:], in_=wvT_d.rearrange("(t p) d -> p t d", p=128))
            nc.sync.dma_start(out=wo_sb[:], in_=woT_d.rearrange("(t p) e -> p t e", p=128))
            nc.sync.dma_start(out=cos_sb[:], in_=cos_d[:])
            nc.sync.dma_start(out=sin_sb[:], in_=sin_d[:])
            nc.sync.dma_start(out=tri_sb[:], in_=tri_d[:])

            # persistent activations
            qT = [persist.tile([128, S], da, tag=f"qT{t}", name=f"qT{t}") for t in range(2)]
            kT = [persist.tile([128, S], da, tag=f"kT{t}", name=f"kT{t}") for t in range(2)]
            outT = [persist.tile([128, S], dt, tag=f"outT{t}", name=f"outT{t}") for t in range(2)]
            vt = [persist.tile([128, GROUP_HEADS, HDIM + 1], da, tag=f"vt{i}", name=f"vt{i}")
                  for i in range(NKT)]

            # ---- Phase A: projections + RoPE (emitted per s-block) ----
            def phase_a(sb):
                ssl = slice(sb * 512, (sb + 1) * 512)
                x_sbs = []
                for eh in range(4):
                    x_q = xin.tile([128, 2, 512], dt, tag=f"x{eh}", name=f"x{eh}_{sb}")
                    nc.sync.dma_start(
                        out=x_q[:],
                        in_=xT_d[eh * 256:(eh + 1) * 256, ssl]
                        .rearrange("(t p) s -> p t s", p=128),
                    )
                    x_sbs.append(x_q)

                def xe(et):
                    return x_sbs[et // 2][:, et % 2, :]

                for dtl in range(2):
                    dsl = slice(dtl * 128, (dtl + 1) * 128)
                    for w_sb, dest in ((wq_sb, qT), (wk_sb, kT)):
                        pp = pv_ps.tile([128, 512], f32, tag="ppv", name=f"pp{sb}{dtl}")
                        for et in range(NET):
                            nc.tensor.matmul(
                                pp[:], w_sb[:, et, dsl], xe(et),
                                start=(et == 0), stop=(et == NET - 1),
                            )
                        # RoPE: r = raw*C + rot(raw)*S
                        raw = rope.tile([128, 512], da, tag="raw")
                        nc.vector.tensor_copy(raw[:], pp[:])
                        rot = rope.tile([128, 512], da, tag="rot")
                        for blk in range(4):
                            srcb = (blk ^ 1) * 32
                            nc.gpsimd.dma_start(
                                out=rot[blk * 32:(blk + 1) * 32, :],
                                in_=raw[srcb:srcb + 32, :],
                            )
                        t1 = rope.tile([128, 512], da, tag="t1")
                        nc.vector.tensor_mul(t1[:], raw[:], cos_sb[:, ssl])
                        t2 = rope.tile([128, 512], da, tag="t2")
                        nc.vector.tensor_mul(t2[:], rot[:], sin_sb[:, ssl])
                        nc.vector.tensor_add(dest[dtl][:, ssl], t1[:], t2[:])
                # v projection (natural layout [s, d_local]) + ones column
                for st in range(4):
                    kt = sb * 4 + st
                    vp = pv_ps.tile([128, DL], f32, tag="ppv", name=f"vp{kt}")
                    for et in range(NET):
                        nc.tensor.matmul(
                            vp[:], xe(et)[:, st * 128:(st + 1) * 128],
                            wv_sb[:, et, :],
                            start=(et == 0), stop=(et == NET - 1),
                        )
                    nc.vector.tensor_copy(
                        vt[kt][:, :, 0:HDIM],
                        vp.rearrange("p (h d) -> p h d", h=GROUP_HEADS),
                    )
                    nc.gpsimd.dma_start(
                        out=vt[kt][:, :, HDIM:HDIM + 1],
                        in_=ones_d.rearrange("p (h o) -> p h o", o=1),
                    )

            # ---- Phase B: head pairs interleaved, PV lags one k-group ----
            def phase_b(qb):
                qsl = slice(qb * 512, (qb + 1) * 512)
                nkt = 4 * (qb + 1)
                for hp in range(2):
                    t = hp
                    pv = [
                        pv_ps.tile([128, 512], f32, tag="ppv", name=f"pv{qb}{hp}{h2}")
                        for h2 in range(2)
                    ]

                    def make_pv_stage(kp, pr, pv=pv, hp=hp):
                        def emit():
                            for h2 in range(2):
                                h = 2 * hp + h2
                                for j in range(2):
                                    kt = 2 * kp + j
                                    o = max(kt * 128 - qb * 512, 0)
                                    if kt >= 4 * qb:
                                        nc.vector.tensor_mul(
                                            pr[h2][:, j * 512 + o:j * 512 + o + 128],
                                            pr[h2][:, j * 512 + o:j * 512 + o + 128],
                                            tri_sb[:],
                                        )
                                    nc.tensor.matmul(
                                        pv[h2][0:65, o:512], vt[kt][:, h, :],
                                        pr[h2][:, j * 512 + o:(j + 1) * 512],
                                        start=(kt == 0), stop=(kt == nkt - 1),
                                    )
                        return emit

                    pv_prev = None
                    for kp in range(nkt // 2):
                        o0 = max(2 * kp * 128 - qb * 512, 0)
                        sc = []
                        for h2 in range(2):
                            psl = slice(h2 * 64, h2 * 64 + 64)
                            s_t = sc_ps.tile([128, 1024], f32, tag="sc")
                            for j in range(2):
                                kt = 2 * kp + j
                                o = max(kt * 128 - qb * 512, 0)
                                nc.tensor.matmul(
                                    s_t[:, j * 512 + o:(j + 1) * 512],
                                    kT[t][psl, kt * 128:(kt + 1) * 128],
                                    qT[t][psl, qb * 512 + o:(qb + 1) * 512],
                                    start=True, stop=True,
                                )
                            sc.append(s_t)
                        pr = []
                        for h2 in range(2):
                            p_t = probs.tile([128, 1024], da, tag="pr")
                            nc.scalar.activation(
                                p_t[:, o0:], sc[h2][:, o0:], Exp, scale=ATTN_SCALE
                            )
                            pr.append(p_t)
                        if pv_prev is not None:
                            pv_prev()
                        pv_prev = make_pv_stage(kp, pr)
                    pv_prev()

                    for h2 in range(2):
                        pvs = small.tile([65, 512], f32, tag="pvs", bufs=2)
                        nc.vector.tensor_copy(pvs[:], pv[h2][0:65, :])
                        sums = small.tile([1, 512], f32, tag="sums")
                        nc.vector.tensor_copy(sums[:], pvs[64:65, :])
                        inv = small.tile([1, 512], f32, tag="inv")
                        nc.vector.reciprocal_approx_fast(out=inv[:], in_=sums[:])
                        invb = small.tile([64, 512], f32, tag="invb")
                        nc.gpsimd.partition_broadcast(invb[:], inv[:])
                        nc.vector.tensor_mul(
                            outT[t][h2 * 64 + 0:h2 * 64 + 64, qsl], pvs[0:64, :], invb[:]
                        )

            # ---- Phase C: output projection (partial over local dims) ----
            def phase_c(qt):
                qsl = slice(qt * 128, (qt + 1) * 128)
                f = sc_ps.tile([128, 1024], f32, tag="fC", name=f"f{qt}", bufs=1)
                for eb in range(2):
                    esl = slice(eb * 512, (eb + 1) * 512)
                    for dtl in range(2):
                        nc.tensor.matmul(
                            f[:, esl], outT[dtl][:, qsl], wo_sb[:, dtl, esl],
                            start=(dtl == 0), stop=(dtl == 1),
                        )
                fo = fopool.tile([128, 1024], f32, tag="fo")
                if qt % 2 == 0:
                    nc.scalar.copy(fo[:], f[:])
                else:
                    nc.vector.tensor_copy(fo[:], f[:])
                nc.sync.dma_start(out=part_d[qsl, :], in_=fo[:])

            # ---- interleaved emission: A(sb) feeds B(qb=sb); C trails B ----
            phase_a(0)
            for blk in range(NQB):
                if blk + 1 < NSB:
                    phase_a(blk + 1)
                phase_b(blk)
                if blk >= 1:
                    for qt in range(4 * (blk - 1), 4 * blk):
                        phase_c(qt)
            for qt in range(4 * (NQB - 1), NQT):
                phase_c(qt)

    nc.compile()
    return nc


def _host_tables():
    inv_freq = 1.0 / (ROPE_BASE ** (np.arange(HALF, dtype=np.float32) / HALF))
    angles = np.arange(S, dtype=np.float32)[:, None] * inv_freq[None, :]  # [S, 32]
    cos = np.cos(angles).T.astype(np.float32)  # [32, S]
    sin = np.sin(angles).T.astype(np.float32)
    cosP = np.tile(cos, (4, 1))                                   # [128, S]
    sinP = np.concatenate([-sin, sin, -sin, sin], axis=0).astype(np.float32)
    tri = (np.arange(128)[None, :] >= np.arange(128)[:, None]).astype(np.float32)
    return cosP, sinP, np.ascontiguousarray(tri)


def kernel(x, Wq, Wk, Wv, Wo):
    import ml_dtypes
    from concourse.bass_utils import run_bass_kernel_spmd

    x = np.asarray(x, dtype=np.float32)
    Wq = np.asarray(Wq, dtype=np.float32)
    Wk = np.asarray(Wk, dtype=np.float32)
    Wv = np.asarray(Wv, dtype=np.float32)
    Wo = np.asarray(Wo, dtype=np.float32)

    mode = DTYPE_MODE
    if ("nc", mode) not in _CACHE:
        _CACHE[("nc", mode)] = _build_nc(mode)
    nc = _CACHE[("nc", mode)]

    np_dt = ml_dtypes.bfloat16 if mode == "bf16" else np.float32
    np_da = np.float32 if mode == "f32r" else ml_dtypes.bfloat16

    def cvt(a):
        return np.ascontiguousarray(a.astype(np_dt))

    def cva(a):
        return np.ascontiguousarray(a.astype(np_da))

    # RoPE pair deinterleave permutation within each head: [0,2,..62, 1,3,..63]
    perm = np.concatenate([np.arange(0, HDIM, 2), np.arange(1, HDIM, 2)])
    full_perm = np.concatenate([h * HDIM + perm for h in range(N_HEADS)])
    Wq_p = Wq[full_perm]
    Wk_p = Wk[full_perm]

    cosP, sinP, tri = _host_tables()
    xT = [cvt(x[b].T) for b in range(B)]
    cosP, sinP, tri = cva(cosP), cva(sinP), cva(tri)
    ones = np.ones((128, GROUP_HEADS), dtype=np_da)

    in_maps = []
    for c in range(N_CORES):
        b, g = c // 4, c % 4
        dsl = slice(g * DL, (g + 1) * DL)
        in_maps.append({
            "xT": xT[b],
            "wqT": cvt(Wq_p[dsl].T),
            "wkT": cvt(Wk_p[dsl].T),
            "wvT": cvt(Wv[dsl].T),
            "woT": cvt(Wo[:, dsl].T),
            "cosP": cosP,
            "sinP": sinP,
            "tri": tri,
            "ones": ones,
        })

    trace = bool(int(os.environ.get("ANT_KERNEL_TRACE", "0")))
    res = None
    for attempt in range(3):
        try:
            res = run_bass_kernel_spmd(
                nc, in_maps, core_ids=list(range(N_CORES)), trace=trace
            )
            break
        except Exception:
            if attempt == 2:
                raise
            import time as _time
            _time.sleep(20)
    _CACHE["last_exec_time_ns"] = res.exec_time_ns
    _CACHE["last_res"] = res

    out = np.zeros((B, S, E), dtype=np.float32)
    for c in range(N_CORES):
        out[c // 4] += res.results[c]["part"]
    return out



# revision 16
# speedup vs baseline: 1.1406x; 1.0284x over previous
"""Trainium2 Bass kernel for CovariateAttention (B=2, S=2048, E=1024, 16 heads).

Sharding: 8 cores = 2 (batch) x 4 (head groups of 4 heads).
Per core: q/k/v projections for its 4 heads (tensor-parallel column shard),
RoPE, causal flash-style attention with transposed scores, output projection
row-shard producing a partial [S, E] result; host sums the 4 partials per batch.

Layout strategy (everything pre-transposed on host so no device transposes):
  xT   [E, S]    - x[b] transposed
  wqT/wkT [E, 256] - per-head-deinterleaved (RoPE pair permutation) W slices, transposed
  wvT  [E, 256]
  woT  [256, E]  - Wo column-slice transposed
  qT/kT on device: [d_local, S] with per-head layout [x1(32) | x2(32)]
  scoresT [k_pos, q_pos] so softmax sums come from a ones-row in the PV matmul.
Matmul operand dtype defaults to float32r (~2e-4 rel err, 241us); set
ANT_KERNEL_DTYPE=bf16 for ~220us at ~3e-3 rel err.
"""

import os
import sys

sys.path.insert(0, "/opt/trn_rl_repo")

import numpy as np

N_HEADS = 16
ROPE_BASE = 10000.0
B, S, E = 2, 2048, 1024
D_ATTN = 1024
HDIM = 64
HALF = HDIM // 2
GROUP_HEADS = 4          # heads per core
DL = GROUP_HEADS * HDIM  # 256 local dims per core
N_CORES = 8
ATTN_SCALE = 1.0 / np.sqrt(D_ATTN)

DTYPE_MODE = os.environ.get("ANT_KERNEL_DTYPE", "bf16")

_CACHE = {}


def _build_nc(mode):
    import concourse.tile as tile
    from concourse import bacc, mybir

    f32 = mybir.dt.float32
    dt = mybir.dt.bfloat16 if mode == "bf16" else mybir.dt.float32r
    da = mybir.dt.float32r if mode == "f32r" else mybir.dt.bfloat16  # attention core
    dt_bits = mybir.dt.uint16 if mode == "bf16" else mybir.dt.uint32

    nc = bacc.Bacc("TRN2", target_bir_lowering=False, debug=False, num_devices=N_CORES)

    xT_d = nc.dram_tensor("xT", [E, S], dt, kind="ExternalInput").ap()
    wqT_d = nc.dram_tensor("wqT", [E, DL], dt, kind="ExternalInput").ap()
    wkT_d = nc.dram_tensor("wkT", [E, DL], dt, kind="ExternalInput").ap()
    wvT_d = nc.dram_tensor("wvT", [E, DL], dt, kind="ExternalInput").ap()
    woT_d = nc.dram_tensor("woT", [DL, E], dt, kind="ExternalInput").ap()
    cos_d = nc.dram_tensor("cosP", [128, S], da, kind="ExternalInput").ap()
    sin_d = nc.dram_tensor("sinP", [128, S], da, kind="ExternalInput").ap()
    tri_d = nc.dram_tensor("tri", [128, 128], da, kind="ExternalInput").ap()
    ones_d = nc.dram_tensor("ones", [128, GROUP_HEADS], da, kind="ExternalInput").ap()
    part_d = nc.dram_tensor("part", [S, E], f32, kind="ExternalOutput").ap()

    NSB = 4    # s-blocks of 512 (projection phase)
    NET = 8    # e-tiles of 128 (contraction)
    NQB = 4    # q-blocks of 512 (attention phase)
    NKT = 16   # k-tiles of 128
    NQT = 16   # q-tiles of 128 (output projection)
    Exp = mybir.ActivationFunctionType.Exp

    with tile.TileContext(nc) as tc:
        with (
            tc.tile_pool(name="weights", bufs=1) as wpool,
            tc.tile_pool(name="persist", bufs=1) as persist,
            tc.tile_pool(name="xin", bufs=2) as xin,
            tc.tile_pool(name="rope", bufs=3 if mode == "bf16" else 2) as rope,
            tc.tile_pool(name="probs", bufs=4 if mode == "bf16" else 3) as probs,
            tc.tile_pool(name="small", bufs=3) as small,
            tc.tile_pool(name="fout", bufs=4) as fopool,
            tc.tile_pool(name="sc_ps", bufs=2, space="PSUM") as sc_ps,
            tc.tile_pool(name="pv_ps", bufs=4, space="PSUM") as pv_ps,
        ):
            # ---- load weights/constants (resident) ----
            wq_sb = wpool.tile([128, NET, DL], dt, tag="wq")
            wk_sb = wpool.tile([128, NET, DL], dt, tag="wk")
            wv_sb = wpool.tile([128, NET, DL], dt, tag="wv")
            wo_sb = wpool.tile([128, 2, E], dt, tag="wo")
            cos_sb = wpool.tile([128, S], da, tag="cos")
            sin_sb = wpool.tile([128, S], da, tag="sin")
            tri_sb = wpool.tile([128, 128], da, tag="tri")
            nc.sync.dma_start(out=wq_sb[:], in_=wqT_d.rearrange("(t p) d -> p t d", p=128))
            nc.sync.dma_start(out=wk_sb[:], in_=wkT_d.rearrange("(t p) d -> p t d", p=128))
            nc.sync.dma_start(out=wv_sb[:], in_=wvT_d.rearrange("(t p) d -> p t d", p=128))
            nc.sync.dma_start(out=wo_sb[:], in_=woT_d.rearrange("(t p) e -> p t e", p=128))
            nc.sync.dma_start(out=cos_sb[:], in_=cos_d[:])
            nc.sync.dma_start(out=sin_sb[:], in_=sin_d[:])
            nc.sync.dma_start(out=tri_sb[:], in_=tri_d[:])

            # persistent activations
            qT = [persist.tile([128, S], da, tag=f"qT{t}", name=f"qT{t}") for t in range(2)]
            kT = [persist.tile([128, S], da, tag=f"kT{t}", name=f"kT{t}") for t in range(2)]
            outT = [persist.tile([128, S], dt, tag=f"outT{t}", name=f"outT{t}") for t in range(2)]
            vt = [persist.tile([128, GROUP_HEADS, HDIM + 1], da, tag=f"vt{i}", name=f"vt{i}")
                  for i in range(NKT)]

            # ---- Phase A: projections + RoPE (emitted per s-block) ----
            def phase_a(sb):
                ssl = slice(sb * 512, (sb + 1) * 512)
                x_sbs = []
                for eh in range(4):
                    x_q = xin.tile([128, 2, 512], dt, tag=f"x{eh}", name=f"x{eh}_{sb}")
                    nc.sync.dma_start(
                        out=x_q[:],
                        in_=xT_d[eh * 256:(eh + 1) * 256, ssl]
                        .rearrange("(t p) s -> p t s", p=128),
                    )
                    x_sbs.append(x_q)

                def xe(et):
                    return x_sbs[et // 2][:, et % 2, :]

                for dtl in range(2):
                    dsl = slice(dtl * 128, (dtl + 1) * 128)
                    for w_sb, dest in ((wq_sb, qT), (wk_sb, kT)):
                        pp = pv_ps.tile([128, 512], f32, tag="ppv", name=f"pp{sb}{dtl}")
                        for et in range(NET):
                            nc.tensor.matmul(
                                pp[:], w_sb[:, et, dsl], xe(et),
                                start=(et == 0), stop=(et == NET - 1),
                            )
                        # RoPE: r = raw*C + rot(raw)*S
                        raw = rope.tile([128, 512], da, tag="raw")
                        nc.vector.tensor_copy(raw[:], pp[:])
                        rot = rope.tile([128, 512], da, tag="rot")
                        for blk in range(4):
                            srcb = (blk ^ 1) * 32
                            nc.gpsimd.dma_start(
                                out=rot[blk * 32:(blk + 1) * 32, :],
                                in_=raw[srcb:srcb + 32, :],
                            )
                        t1 = rope.tile([128, 512], da, tag="t1")
                        nc.vector.tensor_mul(t1[:], raw[:], cos_sb[:, ssl])
                        t2 = rope.tile([128, 512], da, tag="t2")
                        nc.vector.tensor_mul(t2[:], rot[:], sin_sb[:, ssl])
                        nc.vector.tensor_add(dest[dtl][:, ssl], t1[:], t2[:])
                # v projection (natural layout [s, d_local]) + ones column
                for st in range(4):
                    kt = sb * 4 + st
                    vp = pv_ps.tile([128, DL], f32, tag="ppv", name=f"vp{kt}")
                    for et in range(NET):
                        nc.tensor.matmul(
                            vp[:], xe(et)[:, st * 128:(st + 1) * 128],
                            wv_sb[:, et, :],
                            start=(et == 0), stop=(et == NET - 1),
                        )
                    nc.vector.tensor_copy(
                        vt[kt][:, :, 0:HDIM],
                        vp.rearrange("p (h d) -> p h d", h=GROUP_HEADS),
                    )
                    nc.gpsimd.dma_start(
                        out=vt[kt][:, :, HDIM:HDIM + 1],
                        in_=ones_d.rearrange("p (h o) -> p h o", o=1),
                    )

            # ---- Phase B: head pairs interleaved, PV lags one k-group ----
            def phase_b(qb):
                qsl = slice(qb * 512, (qb + 1) * 512)
                nkt = 4 * (qb + 1)
                for hp in range(2):
                    t = hp
                    pv = [
                        pv_ps.tile([128, 512], f32, tag="ppv", name=f"pv{qb}{hp}{h2}")
                        for h2 in range(2)
                    ]

                    def make_pv_stage(kp, pr, pv=pv, hp=hp):
                        def emit():
                            for h2 in range(2):
                                h = 2 * hp + h2
                                for j in range(2):
                                    kt = 2 * kp + j
                                    o = max(kt * 128 - qb * 512, 0)
                                    if kt >= 4 * qb:
                                        nc.vector.tensor_mul(
                                            pr[h2][:, j * 512 + o:j * 512 + o + 128],
                                            pr[h2][:, j * 512 + o:j * 512 + o + 128],
                                            tri_sb[:],
                                        )
                                    nc.tensor.matmul(
                                        pv[h2][0:65, o:512], vt[kt][:, h, :],
                                        pr[h2][:, j * 512 + o:(j + 1) * 512],
                                        start=(kt == 0), stop=(kt == nkt - 1),
                                    )
                        return emit

                    pv_prev = None
                    for kp in range(nkt // 2):
                        o0 = max(2 * kp * 128 - qb * 512, 0)
                        sc = []
                        for h2 in range(2):
                            psl = slice(h2 * 64, h2 * 64 + 64)
                            s_t = sc_ps.tile([128, 1024], f32, tag="sc")
                            for j in range(2):
                                kt = 2 * kp + j
                                o = max(kt * 128 - qb * 512, 0)
                                nc.tensor.matmul(
                                    s_t[:, j * 512 + o:(j + 1) * 512],
                                    kT[t][psl, kt * 128:(kt + 1) * 128],
                                    qT[t][psl, qb * 512 + o:(qb + 1) * 512],
                                    start=True, stop=True,
                                )
                            sc.append(s_t)
                        pr = []
                        for h2 in range(2):
                            p_t = probs.tile([128, 1024], da, tag="pr")
                            nc.scalar.activation(
                                p_t[:, o0:], sc[h2][:, o0:], Exp, scale=ATTN_SCALE
                            )
                            pr.append(p_t)
                        if pv_prev is not None:
                            pv_prev()
                        pv_prev = make_pv_stage(kp, pr)
                    pv_prev()

                    for h2 in range(2):
                        pvs = small.tile([65, 512], f32, tag="pvs", bufs=2)
                        nc.vector.tensor_copy(pvs[:], pv[h2][0:65, :])
                        sums = small.tile([1, 512], f32, tag="sums")
                        nc.vector.tensor_copy(sums[:], pvs[64:65, :])
                        inv = small.tile([1, 512], f32, tag="inv")
                        nc.vector.reciprocal_approx_fast(out=inv[:], in_=sums[:])
                        invb = small.tile([64, 512], f32, tag="invb")
                        nc.gpsimd.partition_broadcast(invb[:], inv[:])
                        nc.vector.tensor_mul(
                            outT[t][h2 * 64 + 0:h2 * 64 + 64, qsl], pvs[0:64, :], invb[:]
                        )

            # ---- Phase C: output projection (partial over local dims) ----
            def phase_c(qt):
                qsl = slice(qt * 128, (qt + 1) * 128)
                for eb in range(2):
                    esl = slice(eb * 512, (eb + 1) * 512)
                    f = pv_ps.tile([128, 512], f32, tag="ppv", name=f"f{qt}{eb}")
                    for dtl in range(2):
                        nc.tensor.matmul(
                            f[:], outT[dtl][:, qsl], wo_sb[:, dtl, esl],
                            start=(dtl == 0), stop=(dtl == 1),
                        )
                    fo = fopool.tile([128, 512], f32, tag="fo")
                    if (qt + eb) % 2 == 0:
                        nc.scalar.copy(fo[:], f[:])
                    else:
                        nc.vector.tensor_copy(fo[:], f[:])
                    nc.sync.dma_start(out=part_d[qsl, esl], in_=fo[:])

            # ---- interleaved emission: A(sb) feeds B(qb=sb); C trails B ----
            phase_a(0)
            for blk in range(NQB):
                if blk + 1 < NSB:
                    phase_a(blk + 1)
                phase_b(blk)
                if blk >= 1:
                    for qt in range(4 * (blk - 1), 4 * blk):
                        phase_c(qt)
            for qt in range(4 * (NQB - 1), NQT):
                phase_c(qt)

    nc.compile()
    return nc


def _host_tables():
    inv_freq = 1.0 / (ROPE_BASE ** (np.arange(HALF, dtype=np.float32) / HALF))
    angles = np.arange(S, dtype=np.float32)[:, None] * inv_freq[None, :]  # [S, 32]
    cos = np.cos(angles).T.astype(np.float32)  # [32, S]
    sin = np.sin(angles).T.astype(np.float32)
    cosP = np.tile(cos, (4, 1))                                   # [128, S]
    sinP = np.concatenate([-sin, sin, -sin, sin], axis=0).astype(np.float32)
    tri = (np.arange(128)[None, :] >= np.arange(128)[:, None]).astype(np.float32)
    return cosP, sinP, np.ascontiguousarray(tri)


def kernel(x, Wq, Wk, Wv, Wo):
    import ml_dtypes
    from concourse.bass_utils import run_bass_kernel_spmd

    x = np.asarray(x, dtype=np.float32)
    Wq = np.asarray(Wq, dtype=np.float32)
    Wk = np.asarray(Wk, dtype=np.float32)
    Wv = np.asarray(Wv, dtype=np.float32)
    Wo = np.asarray(Wo, dtype=np.float32)

    mode = DTYPE_MODE
    if ("nc", mode) not in _CACHE:
        _CACHE[("nc", mode)] = _build_nc(mode)
    nc = _CACHE[("nc", mode)]

    np_dt = ml_dtypes.bfloat16 if mode == "bf16" else np.float32
    np_da = np.float32 if mode == "f32r" else ml_dtypes.bfloat16

    def cvt(a):
        return np.ascontiguousarray(a.astype(np_dt))

    def cva(a):
        return np.ascontiguousarray(a.astype(np_da))

    # RoPE pair deinterleave permutation within each head: [0,2,..62, 1,3,..63]
    perm = np.concatenate([np.arange(0, HDIM, 2), np.arange(1, HDIM, 2)])
    full_perm = np.concatenate([h * HDIM + perm for h in range(N_HEADS)])
    Wq_p = Wq[full_perm]
    Wk_p = Wk[full_perm]

    cosP, sinP, tri = _host_tables()
    xT = [cvt(x[b].T) for b in range(B)]
    cosP, sinP, tri = cva(cosP), cva(sinP), cva(tri)
    ones = np.ones((128, GROUP_HEADS), dtype=np_da)

    in_maps = []
    for c in range(N_CORES):
        b, g = c // 4, c % 4
        dsl = slice(g * DL, (g + 1) * DL)
        in_maps.append({
            "xT": xT[b],
            "wqT": cvt(Wq_p[dsl].T),
            "wkT": cvt(Wk_p[dsl].T),
            "wvT": cvt(Wv[dsl].T),
            "woT": cvt(Wo[:, dsl].T),
            "cosP": cosP,
            "sinP": sinP,
            "tri": tri,
            "ones": ones,
        })

    trace = bool(int(os.environ.get("ANT_KERNEL_TRACE", "0")))
    res = None
    for attempt in range(3):
        try:
            res = run_bass_kernel_spmd(
                nc, in_maps, core_ids=list(range(N_CORES)), trace=trace
            )
            break
        except Exception:
            if attempt == 2:
                raise
            import time as _time
            _time.sleep(20)
    _CACHE["last_exec_time_ns"] = res.exec_time_ns
    _CACHE["last_res"] = res

    out = np.zeros((B, S, E), dtype=np.float32)
    for c in range(N_CORES):
        out[c // 4] += res.results[c]["part"]
    return out

